# revision 20
# baseline (speedup 1.0000x reference)
"""ArcMarginProduct (ArcFace) forward on 8 TRN2 NeuronCores.

out[b, c] = s * cos(theta_bc)         except at c == label[b] where
out[b, c] = s * phi(cos(theta_bc))    (margin epilogue)

Strategy (classification-parallel / Partial-FC), v8:
  - pad C 84281 -> 86016 = 8 * 10752, shard class rows across 8 cores
  - host precomputes xt = bf16((s * x / ||x||).T)  [D, B] and
    wnT = bf16((w / ||w_c||).T)  [D, CS] per shard -> the device kernel
    is a pure bf16 matmul: out^T[c, b] = wnT^T @ xt, PE-bound, with no
    on-device transposes, casts, or normalization
  - margin epilogue (512 scattered label positions) applied on host
  - wnT-load DMAs ride the scalar(Act) HW DGE queue (prefetch depth 2),
    out-store DMAs the sync(SP) queue -> independent streams
  - PSUM->SBUF eviction split: scalar takes 2 of 4 class windows
    (activation Copy), vector the other 2 (tensor_copy)
  - host concatenates shards, drops padding, transposes, casts to f32
"""

import math

import numpy as np

B = 512
D = 512
C = 84281
NCORES = 8
TILE = 512             # classes per tile (4 matmul M-windows of 128)
NT = 21                # tiles per core
CS = NT * TILE         # 10752 padded classes per core
REAL = [10536] * 7 + [C - 10536 * 7]   # real class rows per core
BASE = [10536 * i for i in range(NCORES)]
PF = 3                 # w-DMA prefetch depth in tiles

S_SCALE = 32.0
MARGIN = 0.5
COS_M = math.cos(MARGIN)
SIN_M = math.sin(MARGIN)
TH = math.cos(math.pi - MARGIN)
MM = math.sin(math.pi - MARGIN) * MARGIN

_CACHE = {}


def _build_nc():
    import concourse.tile as tile
    from concourse import bacc, mybir
    from contextlib import ExitStack

    bf16 = mybir.dt.bfloat16
    f32 = mybir.dt.float32

    nc = bacc.Bacc("TRN2", target_bir_lowering=False, debug=False, num_devices=NCORES)
    w_ext = nc.declare_dram_parameter("wnt", [D, CS], bf16, isOutput=False)
    xt_ext = nc.declare_dram_parameter("xt", [D, B], bf16, isOutput=False)
    out_ext = nc.declare_dram_parameter("out", [CS, B], bf16, isOutput=True)

    # class g = t*TILE + m*128 + p  ->  tile t, M-window m, psum partition p
    w_view = w_ext[:].rearrange("(k p) c -> p k c", p=128)          # [128, 4, CS]
    xt_view = xt_ext[:].rearrange("(k p) b -> p k b", p=128)        # [128, 4, B]
    out_view = out_ext[:].rearrange("(t m p) b -> p t m b", p=128, m=4)

    with tile.TileContext(nc) as tc, ExitStack() as es:
        cpool = es.enter_context(tc.tile_pool(name="consts", bufs=1))
        wpool = es.enter_context(tc.tile_pool(name="wch", bufs=5))
        outpool = es.enter_context(tc.tile_pool(name="outch", bufs=3))
        ppool_out = es.enter_context(tc.tile_pool(name="pout", bufs=4, space="PSUM"))

        # ---- w prefetch (scalar HWDGE queue)
        wch_tiles = []

        def issue_w_dma(t):
            wch = wpool.tile([128, 4, TILE], bf16, tag="wch", name="wch")
            if t == 0:
                # split the first load so the very first matmul window can
                # start as soon as its 0.13MB lands
                nc.scalar.dma_start(
                    out=wch[:, :, 0:128], in_=w_view[:, :, 0:128]
                )
                nc.scalar.dma_start(
                    out=wch[:, :, 128:TILE], in_=w_view[:, :, 128:TILE]
                )
            else:
                nc.scalar.dma_start(
                    out=wch[:], in_=w_view[:, :, t * TILE : (t + 1) * TILE]
                )
            wch_tiles.append(wch)

        # ---- one-shot load: xt (pre-normalized, pre-scaled, bf16),
        # one DMA per k slice so each k-accumulation can begin early
        xnT = cpool.tile([128, 4, B], bf16, tag="xnT")
        for k in range(4):
            nc.sync.dma_start(out=xnT[:, k, :], in_=xt_view[:, k, :])

        for t in range(PF):
            issue_w_dma(t)

        def pe(t):
            wch = wch_tiles[t]
            if t + PF < NT:
                issue_w_dma(t + PF)
            pos = []
            for g0 in (0, 2):
                po = ppool_out.tile([128, 2 * B], f32, name="po")
                for jj in range(2):
                    m = g0 + jj
                    for k in range(4):
                        nc.tensor.matmul(
                            po[:, jj * B : (jj + 1) * B],
                            lhsT=wch[:, k, m * 128 : (m + 1) * 128],
                            rhs=xnT[:, k, :],
                            start=(k == 0),
                            stop=(k == 3),
                        )
                pos.append(po)
            return pos

        def outcopy(t, pos, split_store=False):
            """PSUM -> SBUF eviction (scalar m0/m1, vector m2/m3), SP store."""
            outch = outpool.tile([128, 4, B], bf16, tag="outch", name="outch")
            for m in range(4):
                src = pos[m // 2][:, (m % 2) * B : (m % 2 + 1) * B]
                if m < 2:
                    nc.scalar.activation(
                        out=outch[:, m, :],
                        in_=src,
                        func=mybir.ActivationFunctionType.Copy,
                    )
                else:
                    nc.vector.tensor_copy(outch[:, m, :], src)
            if split_store:
                nc.sync.dma_start(out=out_view[:, t, 0:2, :], in_=outch[:, 0:2, :])
                nc.sync.dma_start(out=out_view[:, t, 2:4, :], in_=outch[:, 2:4, :])
            else:
                nc.sync.dma_start(out=out_view[:, t, :, :], in_=outch[:])

        pos_prev = None
        for t in range(NT):
            if pos_prev is not None:
                outcopy(t - 1, pos_prev)
            pos = pe(t)
            pos_prev = pos
        outcopy(NT - 1, pos_prev, split_store=True)

    nc.finalize()
    return nc


def _get_nc():
    if "nc" not in _CACHE:
        _CACHE["nc"] = _build_nc()
    return _CACHE["nc"]


def make_in_maps(x, weight, label):
    import ml_dtypes

    x = np.asarray(x, dtype=np.float32)
    weight = np.asarray(weight, dtype=np.float32)
    xn = x / np.maximum(np.linalg.norm(x, axis=1, keepdims=True), 1e-12)
    xt = np.ascontiguousarray((S_SCALE * xn).T).astype(ml_dtypes.bfloat16)
    wn = weight / np.maximum(
        np.sqrt(np.einsum("cd,cd->c", weight, weight))[:, None], 1e-12
    )
    in_maps = []
    for i in range(NCORES):
        a, r = BASE[i], REAL[i]
        wshard = np.empty((CS, D), dtype=np.float32)
        wshard[:r] = wn[a : a + r]
        wshard[r:] = 1.0
        wnt = np.ascontiguousarray(wshard.T).astype(ml_dtypes.bfloat16)
        in_maps.append({"wnt": wnt, "xt": xt})
    return in_maps


def assemble(results, label):
    shards = [np.asarray(results[i]["out"])[: REAL[i]] for i in range(NCORES)]
    full_t = np.concatenate(shards, axis=0).astype(np.float32)  # [C, B]
    out = np.ascontiguousarray(full_t.T)                        # [B, C]
    # margin epilogue on the 512 label positions
    label = np.asarray(label).astype(np.int64)
    b = np.arange(B)
    cosv = out[b, label] / S_SCALE
    sine = np.sqrt(np.maximum(0.0, 1.0 - cosv * cosv))
    phi = cosv * COS_M - sine * SIN_M
    out[b, label] = np.where(cosv - TH > 0, phi, cosv - MM) * S_SCALE
    return out


def kernel(x, weight, label):
    from concourse.bass_utils import run_bass_kernel_spmd

    nc = _get_nc()
    in_maps = make_in_maps(x, weight, label)
    res = run_bass_kernel_spmd(nc, in_maps, list(range(NCORES)))
    return assemble(res.results, label)


# revision 21
# speedup vs baseline: 1.0145x; 1.0145x over previous
"""ArcMarginProduct (ArcFace) forward on 8 TRN2 NeuronCores.

out[b, c] = s * cos(theta_bc)         except at c == label[b] where
out[b, c] = s * phi(cos(theta_bc))    (margin epilogue)

Strategy (classification-parallel / Partial-FC), v8:
  - pad C 84281 -> 86016 = 8 * 10752, shard class rows across 8 cores
  - host precomputes xt = bf16((s * x / ||x||).T)  [D, B] and
    wnT = bf16((w / ||w_c||).T)  [D, CS] per shard -> the device kernel
    is a pure bf16 matmul: out^T[c, b] = wnT^T @ xt, PE-bound, with no
    on-device transposes, casts, or normalization
  - margin epilogue (512 scattered label positions) applied on host
  - wnT-load DMAs ride the scalar(Act) HW DGE queue (prefetch depth 2),
    out-store DMAs the sync(SP) queue -> independent streams
  - PSUM->SBUF eviction split: scalar takes 2 of 4 class windows
    (activation Copy), vector the other 2 (tensor_copy)
  - host concatenates shards, drops padding, transposes, casts to f32
"""

import math

import numpy as np

B = 512
D = 512
C = 84281
NCORES = 8
TILE = 512             # classes per tile (4 matmul M-windows of 128)
NT = 21                # tiles per core
CS = NT * TILE         # 10752 padded classes per core
REAL = [10536] * 7 + [C - 10536 * 7]   # real class rows per core
BASE = [10536 * i for i in range(NCORES)]
PF = 2                 # w-DMA prefetch depth in tiles

S_SCALE = 32.0
MARGIN = 0.5
COS_M = math.cos(MARGIN)
SIN_M = math.sin(MARGIN)
TH = math.cos(math.pi - MARGIN)
MM = math.sin(math.pi - MARGIN) * MARGIN

_CACHE = {}


def _build_nc():
    import concourse.tile as tile
    from concourse import bacc, mybir
    from contextlib import ExitStack

    bf16 = mybir.dt.bfloat16
    f32 = mybir.dt.float32

    nc = bacc.Bacc("TRN2", target_bir_lowering=False, debug=False, num_devices=NCORES)
    w_ext = nc.declare_dram_parameter("wnt", [D, CS], bf16, isOutput=False)
    xt_ext = nc.declare_dram_parameter("xt", [D, B], bf16, isOutput=False)
    out_ext = nc.declare_dram_parameter("out", [CS, B], bf16, isOutput=True)

    # class g = t*TILE + m*128 + p  ->  tile t, M-window m, psum partition p
    w_view = w_ext[:].rearrange("(k p) c -> p k c", p=128)          # [128, 4, CS]
    xt_view = xt_ext[:].rearrange("(k p) b -> p k b", p=128)        # [128, 4, B]
    out_view = out_ext[:].rearrange("(t m p) b -> p t m b", p=128, m=4)

    with tile.TileContext(nc) as tc, ExitStack() as es:
        cpool = es.enter_context(tc.tile_pool(name="consts", bufs=1))
        wpool = es.enter_context(tc.tile_pool(name="wch", bufs=4))
        outpool = es.enter_context(tc.tile_pool(name="outch", bufs=3))
        ppool_out = es.enter_context(tc.tile_pool(name="pout", bufs=4, space="PSUM"))

        # ---- w prefetch (scalar HWDGE queue)
        wch_tiles = []

        def issue_w_dma(t):
            wch = wpool.tile([128, 4, TILE], bf16, tag="wch", name="wch")
            if t == 0:
                # split the first load so the very first matmul window can
                # start as soon as its 0.13MB lands
                nc.scalar.dma_start(
                    out=wch[:, :, 0:128], in_=w_view[:, :, 0:128]
                )
                nc.scalar.dma_start(
                    out=wch[:, :, 128:TILE], in_=w_view[:, :, 128:TILE]
                )
            else:
                nc.scalar.dma_start(
                    out=wch[:], in_=w_view[:, :, t * TILE : (t + 1) * TILE]
                )
            wch_tiles.append(wch)

        # ---- one-shot load: xt (pre-normalized, pre-scaled, bf16),
        # one DMA per k slice so each k-accumulation can begin early
        xnT = cpool.tile([128, 4, B], bf16, tag="xnT")
        for k in range(4):
            nc.sync.dma_start(out=xnT[:, k, :], in_=xt_view[:, k, :])

        for t in range(PF):
            issue_w_dma(t)

        def pe(t):
            wch = wch_tiles[t]
            if t + PF < NT:
                issue_w_dma(t + PF)
            pos = []
            for g0 in (0, 2):
                po = ppool_out.tile([128, 2 * B], f32, name="po")
                for jj in range(2):
                    m = g0 + jj
                    for k in range(4):
                        nc.tensor.matmul(
                            po[:, jj * B : (jj + 1) * B],
                            lhsT=wch[:, k, m * 128 : (m + 1) * 128],
                            rhs=xnT[:, k, :],
                            start=(k == 0),
                            stop=(k == 3),
                        )
                pos.append(po)
            return pos

        def outcopy(t, pos, split_store=False):
            """PSUM -> SBUF eviction (scalar m0/m1, vector m2/m3), SP store."""
            outch = outpool.tile([128, 4, B], bf16, tag="outch", name="outch")
            for m in range(4):
                src = pos[m // 2][:, (m % 2) * B : (m % 2 + 1) * B]
                if m < 2:
                    nc.scalar.activation(
                        out=outch[:, m, :],
                        in_=src,
                        func=mybir.ActivationFunctionType.Copy,
                    )
                else:
                    nc.vector.tensor_copy(outch[:, m, :], src)
            if split_store:
                nc.sync.dma_start(out=out_view[:, t, 0:2, :], in_=outch[:, 0:2, :])
                nc.sync.dma_start(out=out_view[:, t, 2:4, :], in_=outch[:, 2:4, :])
            else:
                nc.sync.dma_start(out=out_view[:, t, :, :], in_=outch[:])

        pos_prev = None
        for t in range(NT):
            if pos_prev is not None:
                outcopy(t - 1, pos_prev)
            pos = pe(t)
            pos_prev = pos
        outcopy(NT - 1, pos_prev, split_store=True)

    nc.finalize()
    return nc


def _get_nc():
    if "nc" not in _CACHE:
        _CACHE["nc"] = _build_nc()
    return _CACHE["nc"]


def make_in_maps(x, weight, label):
    import ml_dtypes

    x = np.asarray(x, dtype=np.float32)
    weight = np.asarray(weight, dtype=np.float32)
    xn = x / np.maximum(np.linalg.norm(x, axis=1, keepdims=True), 1e-12)
    xt = np.ascontiguousarray((S_SCALE * xn).T).astype(ml_dtypes.bfloat16)
    wn = weight / np.maximum(
        np.sqrt(np.einsum("cd,cd->c", weight, weight))[:, None], 1e-12
    )
    in_maps = []
    for i in range(NCORES):
        a, r = BASE[i], REAL[i]
        wshard = np.empty((CS, D), dtype=np.float32)
        wshard[:r] = wn[a : a + r]
        wshard[r:] = 1.0
        wnt = np.ascontiguousarray(wshard.T).astype(ml_dtypes.bfloat16)
        in_maps.append({"wnt": wnt, "xt": xt})
    return in_maps


def assemble(results, label):
    shards = [np.asarray(results[i]["out"])[: REAL[i]] for i in range(NCORES)]
    full_t = np.concatenate(shards, axis=0).astype(np.float32)  # [C, B]
    out = np.ascontiguousarray(full_t.T)                        # [B, C]
    # margin epilogue on the 512 label positions
    label = np.asarray(label).astype(np.int64)
    b = np.arange(B)
    cosv = out[b, label] / S_SCALE
    sine = np.sqrt(np.maximum(0.0, 1.0 - cosv * cosv))
    phi = cosv * COS_M - sine * SIN_M
    out[b, label] = np.where(cosv - TH > 0, phi, cosv - MM) * S_SCALE
    return out


def kernel(x, weight, label):
    from concourse.bass_utils import run_bass_kernel_spmd

    nc = _get_nc()
    in_maps = make_in_maps(x, weight, label)
    res = run_bass_kernel_spmd(nc, in_maps, list(range(NCORES)))
    return assemble(res.results, label)
